# revision 7
# baseline (speedup 1.0000x reference)
"""Trainium2 Bass kernel for CrossAttention (LayerNorm + self-attention + 1x1 conv + residual).

Sharding: data-parallel over batch - B=8, one batch element per NeuronCore.

Per-core design (ScalarE/exp is the roofline: H*L^2 = 16.8M exps @ 1 elem/cycle/lane):
 - all matmuls in fp16 (1 cycle/row on PE vs 4 for fp32), fp32 PSUM accumulation
 - LayerNorm folded into QKV projections via augmented contraction rows
 - softmax without max-subtraction (logits are O(1)); denominator via ones-matmul
 - scores computed in two [128,1024] PSUM tiles (2 banks each) per (d,e) block,
   ping-ponged so the exp ACTIVATEs on ScalarE run back-to-back with no gaps;
   Z/AV matmuls for block e are emitted one position late so the PE never blocks
   the ACT stream; per-d normalize/out-proj tails are pipelined into the next
   d-block's first two positions.
"""
import numpy as np

B, C, L = 8, 256, 2048
H, DH = 4, 32
HID = H * DH           # 128
EPS = 1e-5
SCALE = DH ** -0.5
P = 128                # partitions
DQ = 512               # query tile (free dim of score matmuls)
ND = L // DQ           # 4 d-tiles
NE = L // P            # 16 key tiles of 128

_cached = None


def _patch_act_tables():
    """Steer the greedy ACT-table-load inserter to the combined ln+exp set.

    The inserter picks the first act_func_set containing the needed function;
    'exp' hits exp_and_others and 'ln' hits natural_log, so an interleaved
    ln/exp sequence reloads tables on every switch (1.28us each). Dropping
    those functions from the single-function sets (indices preserved) makes
    both resolve to natural_log_exp_and_others -> one load for the whole
    kernel. Set ids are positional, so only contents may be edited.
    """
    import concourse.bacc as bacc

    if getattr(bacc, '_act_tables_patched', False):
        return
    orig = bacc.get_activation_tables

    def patched(arch):
        tables = {k: set(v) for k, v in orig(arch).items()}
        if 'natural_log_exp_and_others' in tables:
            combined = tables['natural_log_exp_and_others']
            for name, fns in tables.items():
                if name != 'natural_log_exp_and_others':
                    fns -= {f for f in fns if f in combined and str(f).lower().endswith(('exp', 'ln'))}
        return tables

    bacc.get_activation_tables = patched
    bacc._act_tables_patched = True


def _build():
    import concourse.bass as bass
    import concourse.bacc as bacc
    import concourse.tile as tile
    from concourse import mybir
    from concourse.masks import make_identity

    _patch_act_tables()

    f32 = mybir.dt.float32
    f16 = mybir.dt.float16
    AF = mybir.ActivationFunctionType
    OP = mybir.AluOpType

    nc = bacc.Bacc('TRN2', target_bir_lowering=False, debug=False, num_devices=B)

    xd = nc.dram_tensor('x', [C, L], f32, kind='ExternalInput').ap()
    gd = nc.dram_tensor('g', [C, 1], f32, kind='ExternalInput').ap()
    bd = nc.dram_tensor('b', [C, 1], f32, kind='ExternalInput').ap()
    wqd = nc.dram_tensor('Wq', [HID, C], f32, kind='ExternalInput').ap()
    wkd = nc.dram_tensor('Wk', [HID, C], f32, kind='ExternalInput').ap()
    wvd = nc.dram_tensor('Wv', [HID, C], f32, kind='ExternalInput').ap()
    wod = nc.dram_tensor('Wo', [C, HID], f32, kind='ExternalInput').ap()
    bod = nc.dram_tensor('bo', [C, 1], f32, kind='ExternalInput').ap()
    yd = nc.dram_tensor('y', [C, L], f32, kind='ExternalOutput').ap()

    with tile.TileContext(nc) as tc:
        with (
            tc.tile_pool(name='const', bufs=1) as const,
            tc.tile_pool(name='big', bufs=1) as big,
            tc.tile_pool(name='sc', bufs=2) as sc,
            tc.tile_pool(name='apool', bufs=4) as apool,
            tc.tile_pool(name='tpool', bufs=2) as tpool,
            tc.tile_pool(name='psA', bufs=2, space='PSUM') as psA,
            tc.tile_pool(name='psB', bufs=2, space='PSUM') as psB,
            tc.tile_pool(name='psC', bufs=2, space='PSUM') as psC,
        ):
            # ---- input loads (x in 512-col chunks so LN can start early) ----
            x0 = big.tile([P, L], f32, tag='x0')
            x1 = big.tile([P, L], f32, tag='x1')
            for n in range(4):
                sl = slice(n * 512, (n + 1) * 512)
                nc.sync.dma_start(out=x0[:, sl], in_=xd[0:P, sl])
                nc.sync.dma_start(out=x1[:, sl], in_=xd[P:C, sl])

            wq_nat = const.tile([HID, C], f32, tag='wq_nat')
            wk_nat = const.tile([HID, C], f32, tag='wk_nat')
            wv_nat = const.tile([HID, C], f32, tag='wv_nat')
            wo_nat = [const.tile([P, HID], f32, tag=f'wo_nat{c}', name=f'wo_nat{c}') for c in range(2)]
            nc.sync.dma_start(out=wq_nat, in_=wqd)
            nc.sync.dma_start(out=wk_nat, in_=wkd)
            nc.sync.dma_start(out=wv_nat, in_=wvd)
            for c in range(2):
                nc.sync.dma_start(out=wo_nat[c], in_=wod[c * P:(c + 1) * P, :])

            gc = [const.tile([P, 1], f32, tag=f'g{c}', name=f'g{c}') for c in range(2)]
            bc = [const.tile([P, 1], f32, tag=f'b{c}', name=f'b{c}') for c in range(2)]
            boc = [const.tile([P, 1], f32, tag=f'bo{c}', name=f'bo{c}') for c in range(2)]
            for c in range(2):
                nc.sync.dma_start(out=gc[c], in_=gd[c * P:(c + 1) * P, :])
                nc.sync.dma_start(out=bc[c], in_=bd[c * P:(c + 1) * P, :])
                nc.sync.dma_start(out=boc[c], in_=bod[c * P:(c + 1) * P, :])

            ident = const.tile([P, P], f32, tag='ident')
            make_identity(nc, ident)
            ones16 = const.tile([P, P], f16, tag='ones16')
            nc.gpsimd.memset(ones16, 1.0)
            epst = const.tile([P, 1], f32, tag='epst')
            nc.vector.memset(epst, EPS)

            # ---- weight prep: wT = (W scaled by g)^T in fp16, aug rows ----
            wT = {}
            for name, nat in (('q', wq_nat), ('k', wk_nat), ('v', wv_nat)):
                for c in range(2):
                    tp = psC.tile([P, P], f32, tag='psC')
                    nc.tensor.transpose(tp, nat[:, c * P:(c + 1) * P], ident)
                    t = const.tile([P, HID], f16, tag=f'w{name}T{c}', name=f'w{name}T{c}')
                    nc.vector.tensor_copy(t, tp)
                    wT[(name, c)] = t
            woT = const.tile([HID, C], f16, tag='woT')
            for c in range(2):
                tp = psC.tile([P, P], f32, tag='psC')
                nc.tensor.transpose(tp, wo_nat[c], ident)
                nc.vector.tensor_copy(woT[:, c * P:(c + 1) * P], tp)

            # augmentation rows: row0 = -s_g (for mean), row1 = b_proj
            augin = []
            for c in range(2):
                ai = const.tile([P, 2], f16, tag=f'augin{c}', name=f'augin{c}')
                nc.vector.tensor_scalar_mul(ai[:, 0:1], gc[c], -1.0)
                nc.vector.tensor_copy(ai[:, 1:2], bc[c])
                augin.append(ai)
            augT = {}
            for name in ('q', 'k', 'v'):
                ap_ = psC.tile([2, P], f32, tag='psC', name=f'augps{name}')
                for c in range(2):
                    nc.tensor.matmul(ap_, lhsT=augin[c], rhs=wT[(name, c)],
                                     start=(c == 0), stop=(c == 1))
                t = const.tile([2, P], f16, tag=f'augT{name}', name=f'augT{name}')
                nc.vector.tensor_copy(t, ap_)
                augT[name] = t
            for name in ('q', 'k', 'v'):
                for c in range(2):
                    nc.vector.tensor_scalar_mul(wT[(name, c)], wT[(name, c)], gc[c])

            # ---- LayerNorm stats via fp16 ones-matmuls (replicated on 128 parts) ----
            mean_bc = big.tile([P, L], f32, tag='mean')
            rstd_bc = big.tile([P, L], f32, tag='rstd')
            xs0 = big.tile([P, L], f16, tag='xs0')
            xs1 = big.tile([P, L], f16, tag='xs1')
            aug2 = big.tile([2, L], f16, tag='aug2')
            nc.gpsimd.memset(aug2, 1.0)

            lnv = []
            for hl in range(2):
                hsl = slice(hl * 1024, (hl + 1) * 1024)
                s1h = psA.tile([P, 1024], f32, tag='psA')
                s2h = psA.tile([P, 1024], f32, tag='psA')
                for q2 in range(2):
                    sl = slice(hl * 1024 + q2 * 512, hl * 1024 + (q2 + 1) * 512)
                    psl = slice(q2 * 512, (q2 + 1) * 512)
                    for c, xc in ((0, x0), (1, x1)):
                        xb = sc.tile([P, 512], f16, tag='xb')
                        nc.vector.tensor_copy(xb, xc[:, sl])
                        xsq = sc.tile([P, 512], f16, tag='xsq')
                        nc.gpsimd.tensor_mul(xsq, xc[:, sl], xc[:, sl])
                        nc.tensor.matmul(s1h[:, psl], lhsT=ones16, rhs=xb,
                                         start=(c == 0), stop=(c == 1))
                        nc.tensor.matmul(s2h[:, psl], lhsT=ones16, rhs=xsq,
                                         start=(c == 0), stop=(c == 1))
                nc.vector.tensor_scalar_mul(mean_bc[:, hsl], s1h, 1.0 / C)
                msq = sc.tile([P, 1024], f32, tag='msq')
                nc.vector.tensor_mul(msq, mean_bc[:, hsl], mean_bc[:, hsl])
                veps = sc.tile([P, 1024], f32, tag='veps')
                nc.vector.scalar_tensor_tensor(veps, in0=s2h, scalar=1.0 / C, in1=msq,
                                               op0=OP.mult, op1=OP.subtract)
                lv = sc.tile([P, 1024], f32, tag=f'lnv{hl}', name=f'lnv{hl}')
                nc.scalar.activation(lv, veps, AF.Ln, bias=epst)
                lnv.append(lv)
            for hl in range(2):
                hsl = slice(hl * 1024, (hl + 1) * 1024)
                # rstd = exp(-0.5*ln(var+eps)); keeps ACT in the ln/exp table set
                nc.scalar.activation(rstd_bc[:, hsl], lnv[hl], AF.Exp, scale=-0.5)
                nc.vector.tensor_mul(xs0[:, hsl], x0[:, hsl], rstd_bc[:, hsl])
                nc.vector.tensor_mul(xs1[:, hsl], x1[:, hsl], rstd_bc[:, hsl])
                nc.vector.tensor_mul(aug2[0:1, hsl], mean_bc[0:1, hsl], rstd_bc[0:1, hsl])

            # ---- QKV projections: qt/kt feature-major [HID, L], v position-major ----
            qt = big.tile([HID, L], f16, tag='qt')
            kt = big.tile([HID, L], f16, tag='kt')
            for name, dst in (('q', qt), ('k', kt)):
                for hl in range(2):
                    hsl = slice(hl * 1024, (hl + 1) * 1024)
                    pp = psA.tile([P, 1024], f32, tag='psA')
                    for q2 in range(2):
                        sl = slice(hl * 1024 + q2 * 512, hl * 1024 + (q2 + 1) * 512)
                        psl = slice(q2 * 512, (q2 + 1) * 512)
                        nc.tensor.matmul(pp[:, psl], lhsT=wT[(name, 0)], rhs=xs0[:, sl], start=True, stop=False)
                        nc.tensor.matmul(pp[:, psl], lhsT=wT[(name, 1)], rhs=xs1[:, sl], start=False, stop=False)
                        nc.tensor.matmul(pp[:, psl], lhsT=augT[name], rhs=aug2[:, sl], start=False, stop=True)
                    nc.vector.tensor_copy(dst[:, hsl], pp)
            vsb = big.tile([P, NE, HID], f16, tag='vsb')
            for e in range(NE):
                se = slice(e * P, (e + 1) * P)
                vp = psC.tile([P, HID], f32, tag='psC')
                nc.tensor.matmul(vp, lhsT=xs0[:, se], rhs=wT[('v', 0)], start=True, stop=False)
                nc.tensor.matmul(vp, lhsT=xs1[:, se], rhs=wT[('v', 1)], start=False, stop=False)
                nc.tensor.matmul(vp, lhsT=aug2[:, se], rhs=augT['v'], start=False, stop=True)
                nc.vector.tensor_copy(vsb[:, e, :], vp)

            # ---- attention: pipelined over (d, e); ACT exp stream is the roofline ----
            def emit_qk_exp(d, e):
                sd = slice(d * DQ, (d + 1) * DQ)
                se = slice(e * P, (e + 1) * P)
                # all 4 QK matmuls back-to-back: disjoint 32-row groups and
                # disjoint PSUM banks, so the PE overlaps them (~1 MM span)
                sps = [psA.tile([P, 1024], f32, tag='psA', name=f'sp{d}_{e}_{i}')
                       for i in range(2)]
                for h in range(H):
                    hp = slice(32 * h, 32 * h + 32)
                    nc.tensor.matmul(sps[h // 2][:, (h % 2) * DQ:(h % 2 + 1) * DQ],
                                     lhsT=kt[hp, se], rhs=qt[hp, sd],
                                     start=True, stop=True, tile_position=(32 * h, 0))
                ats = []
                for half in range(2):
                    at = apool.tile([P, 1024], f16, tag='at')
                    nc.scalar.activation(at, sps[half], AF.Exp, scale=SCALE)
                    ats.append(at)
                return ats

            def emit_zav(d, e, ats, zp, op_):
                # phase-grouped: 4 Z matmuls (disjoint 32-col groups) overlap,
                # then 4 AV matmuls overlap
                for dst, lhs in ((zp, None), (op_, vsb)):
                    for h in range(H):
                        hp = slice(32 * h, 32 * h + 32)
                        asl = slice((h % 2) * DQ, (h % 2 + 1) * DQ)
                        lhsT = ones16[:, 0:32] if lhs is None else vsb[:, e, hp]
                        nc.tensor.matmul(dst[hp, :], lhsT=lhsT, rhs=ats[h // 2][:, asl],
                                         start=(e == 0), stop=(e == NE - 1),
                                         tile_position=(0, 32 * h))

            def emit_norm_tail(d, zp, op_):
                rz = tpool.tile([P, DQ], f32, tag='rz')
                nc.vector.reciprocal(rz, zp)
                onorm = tpool.tile([P, DQ], f16, tag='onorm')
                nc.vector.tensor_mul(onorm, op_, rz)
                return onorm

            def emit_proj_tail(d, onorm):
                sd = slice(d * DQ, (d + 1) * DQ)
                for c in range(2):
                    yp = psC.tile([P, DQ], f32, tag='psC')
                    nc.tensor.matmul(yp, lhsT=woT[:, c * P:(c + 1) * P], rhs=onorm,
                                     start=True, stop=True)
                    ysb = tpool.tile([P, DQ], f32, tag='ysb')
                    xc = x0 if c == 0 else x1
                    nc.vector.scalar_tensor_tensor(ysb, in0=yp, scalar=boc[c], in1=xc[:, sd],
                                                   op0=OP.add, op1=OP.add)
                    nc.sync.dma_start(out=yd[c * P:(c + 1) * P, sd], in_=ysb)

            prev = None          # (d, e, ats) whose ZAV is emitted one position late
            zp = op_ = None      # PSUM accumulators of prev's d
            pending_norm = None  # (d, onorm) waiting for its out-projection
            for d in range(ND):
                for e in range(NE):
                    ats = emit_qk_exp(d, e)
                    if prev is not None:
                        emit_zav(prev[0], prev[1], prev[2], zp, op_)
                        if prev[1] == NE - 1:
                            pending_norm = (prev[0], emit_norm_tail(prev[0], zp, op_))
                    if e == 0:
                        # allocate this d's accumulators AFTER the previous d's
                        # reciprocal was emitted (correct WAR ordering on psB)
                        zp = psB.tile([P, DQ], f32, tag='acc', name=f'zp{d}')
                        op_ = psB.tile([P, DQ], f32, tag='acc', name=f'op{d}')
                    elif pending_norm is not None:
                        emit_proj_tail(pending_norm[0], pending_norm[1])
                        pending_norm = None
                    prev = (d, e, ats)
            # drain
            emit_zav(prev[0], prev[1], prev[2], zp, op_)
            onorm_last = emit_norm_tail(ND - 1, zp, op_)
            emit_proj_tail(ND - 1, onorm_last)

    nc.compile()
    return nc


def _get_nc():
    global _cached
    if _cached is None:
        _cached = _build()
    return _cached


def kernel(**inputs):
    from concourse.bass_utils import run_bass_kernel_spmd

    x = np.ascontiguousarray(np.asarray(inputs['x'], dtype=np.float32))
    g = np.asarray(inputs['g'], dtype=np.float32).reshape(C, 1)
    b = np.asarray(inputs['b'], dtype=np.float32).reshape(C, 1)
    wq = np.ascontiguousarray(np.asarray(inputs['Wq'], dtype=np.float32))
    wk = np.ascontiguousarray(np.asarray(inputs['Wk'], dtype=np.float32))
    wv = np.ascontiguousarray(np.asarray(inputs['Wv'], dtype=np.float32))
    wo = np.ascontiguousarray(np.asarray(inputs['Wo'], dtype=np.float32))
    bo = np.asarray(inputs['bo'], dtype=np.float32).reshape(C, 1)

    nc = _get_nc()
    in_maps = [
        {'x': x[i], 'g': g, 'b': b, 'Wq': wq, 'Wk': wk, 'Wv': wv, 'Wo': wo, 'bo': bo}
        for i in range(B)
    ]
    res = run_bass_kernel_spmd(nc, in_maps, list(range(B)))
    return np.stack([res.results[i]['y'] for i in range(B)]).astype(np.float32)


# revision 11
# speedup vs baseline: 1.2405x; 1.2405x over previous
"""Trainium2 Bass kernel for CrossAttention (LayerNorm + self-attention + 1x1 conv + residual).

Sharding: data-parallel over batch - B=8, one batch element per NeuronCore.

Per-core design:
 - ScalarE exp stream is the roofline (H*L^2 = 16.8M exps @ 1 elem/cycle/lane
   ~= 133us); everything else is pipelined underneath it.
 - all matmuls fp16 (1 cycle/row on PE), fp32 PSUM accumulation.
 - LayerNorm folded into QKV projections via augmented contraction rows;
   softmax without max-subtraction (logits are O(1)).
 - scores for 2 heads x 512 queries per [128,1024] PSUM tile, 3-deep rotation;
   Z/AV matmuls run one pipeline position behind the exps.
 - a dead ~5us chained-matmul warmup burst runs during the LayerNorm DVE/ACT
   chain so the PE HAM clock-gate opens (1.2 -> 2.4 GHz) before the attention
   stream starts; the attention keeps PE gaps well under the ~3.4us MID window
   so it never re-throttles.
 - single ACT table load: the ln/exp loads are steered to the combined
   natural_log_exp_and_others set (see _patch_act_tables).
"""
import numpy as np

B, C, L = 8, 256, 2048
H, DH = 4, 32
HID = H * DH           # 128
EPS = 1e-5
SCALE = DH ** -0.5
P = 128                # partitions
DQ = 512               # query tile (free dim of score matmuls)
ND = L // DQ           # 4 d-tiles
NE = L // P            # 16 key tiles of 128

_cached = None


def _patch_act_tables():
    """Steer the greedy ACT-table-load inserter to the combined ln+exp set.

    The inserter picks the first act_func_set containing the needed function;
    'exp' hits exp_and_others and 'ln' hits natural_log, so an interleaved
    ln/exp sequence reloads tables on every switch (1.28us each). Dropping
    those functions from the other sets (indices preserved) makes both
    resolve to natural_log_exp_and_others -> one load for the whole kernel.
    Set ids are positional, so only contents may be edited.
    """
    import concourse.bacc as bacc

    if getattr(bacc, '_act_tables_patched', False):
        return
    orig = bacc.get_activation_tables

    def patched(arch):
        tables = {k: set(v) for k, v in orig(arch).items()}
        if 'natural_log_exp_and_others' in tables:
            combined = tables['natural_log_exp_and_others']
            for name, fns in tables.items():
                if name != 'natural_log_exp_and_others':
                    fns -= {f for f in fns if f in combined and str(f).lower().endswith(('exp', 'ln'))}
        return tables

    bacc.get_activation_tables = patched
    bacc._act_tables_patched = True


def _build():
    import concourse.bass as bass
    import concourse.bacc as bacc
    import concourse.tile as tile
    from concourse import mybir
    from concourse.masks import make_identity

    _patch_act_tables()

    f32 = mybir.dt.float32
    f16 = mybir.dt.float16
    AF = mybir.ActivationFunctionType
    OP = mybir.AluOpType

    nc = bacc.Bacc('TRN2', target_bir_lowering=False, debug=False, num_devices=B)

    xd = nc.dram_tensor('x', [C, L], f32, kind='ExternalInput').ap()
    gd = nc.dram_tensor('g', [C, 1], f32, kind='ExternalInput').ap()
    bd = nc.dram_tensor('b', [C, 1], f32, kind='ExternalInput').ap()
    wqd = nc.dram_tensor('Wq', [HID, C], f32, kind='ExternalInput').ap()
    wkd = nc.dram_tensor('Wk', [HID, C], f32, kind='ExternalInput').ap()
    wvd = nc.dram_tensor('Wv', [HID, C], f32, kind='ExternalInput').ap()
    wod = nc.dram_tensor('Wo', [C, HID], f32, kind='ExternalInput').ap()
    bod = nc.dram_tensor('bo', [C, 1], f32, kind='ExternalInput').ap()
    yd = nc.dram_tensor('y', [C, L], f32, kind='ExternalOutput').ap()
    wsd = nc.dram_tensor('wscr', [1, 1], f32, kind='ExternalOutput').ap()

    with tile.TileContext(nc) as tc:
        with (
            tc.tile_pool(name='const', bufs=1) as const,
            tc.tile_pool(name='big', bufs=1) as big,
            tc.tile_pool(name='sc', bufs=2) as sc,
            tc.tile_pool(name='apool', bufs=4) as apool,
            tc.tile_pool(name='tpool', bufs=2) as tpool,
            tc.tile_pool(name='psA', bufs=3, space='PSUM') as psA,
            tc.tile_pool(name='psB', bufs=2, space='PSUM') as psB,
        ):
            # ---- input loads (x in 512-col chunks so LN can start early) ----
            x0 = big.tile([P, L], f32, tag='x0')
            x1 = big.tile([P, L], f32, tag='x1')
            for n in range(4):
                sl = slice(n * 512, (n + 1) * 512)
                nc.sync.dma_start(out=x0[:, sl], in_=xd[0:P, sl])
                nc.sync.dma_start(out=x1[:, sl], in_=xd[P:C, sl])

            wq_nat = const.tile([HID, C], f32, tag='wq_nat')
            wk_nat = const.tile([HID, C], f32, tag='wk_nat')
            wv_nat = const.tile([HID, C], f32, tag='wv_nat')
            wo_nat = [const.tile([P, HID], f32, tag=f'wo_nat{c}', name=f'wo_nat{c}') for c in range(2)]
            nc.sync.dma_start(out=wq_nat, in_=wqd)
            nc.sync.dma_start(out=wk_nat, in_=wkd)
            nc.sync.dma_start(out=wv_nat, in_=wvd)
            for c in range(2):
                nc.sync.dma_start(out=wo_nat[c], in_=wod[c * P:(c + 1) * P, :])

            gc = [const.tile([P, 1], f32, tag=f'g{c}', name=f'g{c}') for c in range(2)]
            bc = [const.tile([P, 1], f32, tag=f'b{c}', name=f'b{c}') for c in range(2)]
            boc = [const.tile([P, 1], f32, tag=f'bo{c}', name=f'bo{c}') for c in range(2)]
            for c in range(2):
                nc.sync.dma_start(out=gc[c], in_=gd[c * P:(c + 1) * P, :])
                nc.sync.dma_start(out=bc[c], in_=bd[c * P:(c + 1) * P, :])
                nc.sync.dma_start(out=boc[c], in_=bod[c * P:(c + 1) * P, :])

            ident = const.tile([P, P], f32, tag='ident')
            make_identity(nc, ident)
            ones16 = const.tile([P, P], f16, tag='ones16')
            nc.gpsimd.memset(ones16, 1.0)
            epst = const.tile([P, 1], f32, tag='epst')
            nc.vector.memset(epst, EPS)

            # ---- weight prep: wT = (W scaled by g)^T in fp16, aug rows ----
            wT = {}
            for name, nat in (('q', wq_nat), ('k', wk_nat), ('v', wv_nat)):
                for c in range(2):
                    tp = psA.tile([P, P], f32, tag='psA', name=f'tp_{name}{c}')
                    nc.tensor.transpose(tp, nat[:, c * P:(c + 1) * P], ident)
                    t = const.tile([P, HID], f16, tag=f'w{name}T{c}', name=f'w{name}T{c}')
                    nc.vector.tensor_copy(t, tp)
                    wT[(name, c)] = t
            woT = const.tile([HID, C], f16, tag='woT')
            for c in range(2):
                tp = psA.tile([P, P], f32, tag='psA', name=f'tp_wo{c}')
                nc.tensor.transpose(tp, wo_nat[c], ident)
                nc.vector.tensor_copy(woT[:, c * P:(c + 1) * P], tp)

            # augmentation rows: row0 = -s_g (for mean), row1 = b_proj
            augin = []
            for c in range(2):
                ai = const.tile([P, 2], f16, tag=f'augin{c}', name=f'augin{c}')
                nc.vector.tensor_scalar_mul(ai[:, 0:1], gc[c], -1.0)
                nc.vector.tensor_copy(ai[:, 1:2], bc[c])
                augin.append(ai)
            augT = {}
            for name in ('q', 'k', 'v'):
                ap_ = psA.tile([2, P], f32, tag='psA', name=f'augps{name}')
                for c in range(2):
                    nc.tensor.matmul(ap_, lhsT=augin[c], rhs=wT[(name, c)],
                                     start=(c == 0), stop=(c == 1))
                t = const.tile([2, P], f16, tag=f'augT{name}', name=f'augT{name}')
                nc.vector.tensor_copy(t, ap_)
                augT[name] = t
            for name in ('q', 'k', 'v'):
                for c in range(2):
                    nc.vector.tensor_scalar_mul(wT[(name, c)], wT[(name, c)], gc[c])

            # ---- LayerNorm stats via fp16 ones-matmuls (replicated on 128 parts) ----
            mean_bc = big.tile([P, L], f32, tag='mean')
            rstd_bc = big.tile([P, L], f32, tag='rstd')
            xs0 = big.tile([P, L], f16, tag='xs0')
            xs1 = big.tile([P, L], f16, tag='xs1')
            aug2 = big.tile([2, L], f16, tag='aug2')
            nc.gpsimd.memset(aug2, 1.0)

            warm_src = sc.tile([P, 512], f16, tag='wsrc')
            for hl in range(2):
                hsl = slice(hl * 1024, (hl + 1) * 1024)
                s1h = psA.tile([P, 1024], f32, tag='psA', name=f's1h{hl}')
                s2h = psA.tile([P, 1024], f32, tag='psA', name=f's2h{hl}')
                for q2 in range(2):
                    sl = slice(hl * 1024 + q2 * 512, hl * 1024 + (q2 + 1) * 512)
                    psl = slice(q2 * 512, (q2 + 1) * 512)
                    for c, xc in ((0, x0), (1, x1)):
                        xb = sc.tile([P, 512], f16, tag='xb')
                        nc.vector.tensor_copy(xb, xc[:, sl])
                        xsq = sc.tile([P, 512], f16, tag='xsq')
                        nc.gpsimd.tensor_mul(xsq, xc[:, sl], xc[:, sl])
                        nc.tensor.matmul(s1h[:, psl], lhsT=ones16, rhs=xb,
                                         start=(c == 0), stop=(c == 1))
                        nc.tensor.matmul(s2h[:, psl], lhsT=ones16, rhs=xsq,
                                         start=(c == 0), stop=(c == 1))
                if hl == 0:
                    nc.vector.tensor_copy(warm_src, s1h[:, 0:512])
                nc.vector.tensor_scalar_mul(mean_bc[:, hsl], s1h, 1.0 / C)
                msq = sc.tile([P, 1024], f32, tag='msq')
                nc.vector.tensor_mul(msq, mean_bc[:, hsl], mean_bc[:, hsl])
                veps = sc.tile([P, 1024], f32, tag='veps')
                nc.vector.scalar_tensor_tensor(veps, in0=s2h, scalar=1.0 / C, in1=msq,
                                               op0=OP.mult, op1=OP.subtract)
                lv = sc.tile([P, 1024], f32, tag=f'lnv{hl}', name=f'lnv{hl}')
                nc.scalar.activation(lv, veps, AF.Ln, bias=epst)
                # rstd = exp(-0.5*ln(var+eps)); single combined ln/exp ACT table
                nc.scalar.activation(rstd_bc[:, hsl], lv, AF.Exp, scale=-0.5)
                nc.vector.tensor_mul(xs0[:, hsl], x0[:, hsl], rstd_bc[:, hsl])
                nc.vector.tensor_mul(xs1[:, hsl], x1[:, hsl], rstd_bc[:, hsl])
                nc.vector.tensor_mul(aug2[0:1, hsl], mean_bc[0:1, hsl], rstd_bc[0:1, hsl])

            # ---- PE warmup: ~5us of chained dead matmuls during the LN chain
            # so the HAM clock gate opens before the QKV/attention stream ----
            warm_ps = psB.tile([P, 512], f32, tag='acc', name='warm_ps')
            NWARM = 10
            for i in range(NWARM):
                nc.tensor.matmul(warm_ps, lhsT=ones16, rhs=warm_src,
                                 start=(i == 0), stop=(i == NWARM - 1))
            wscr_sb = sc.tile([1, 1], f32, tag='wscr_sb')
            nc.vector.tensor_copy(wscr_sb, warm_ps[0:1, 0:1])
            nc.sync.dma_start(out=wsd, in_=wscr_sb)

            # ---- QKV projections: qt/kt feature-major [HID, L] ----
            qt = big.tile([HID, L], f16, tag='qt')
            kt = big.tile([HID, L], f16, tag='kt')
            for hl in range(2):
                hsl = slice(hl * 1024, (hl + 1) * 1024)
                for name, dst in (('q', qt), ('k', kt)):
                    pp = psA.tile([P, 1024], f32, tag='psA', name=f'pp{name}{hl}')
                    for q2 in range(2):
                        sl = slice(hl * 1024 + q2 * 512, hl * 1024 + (q2 + 1) * 512)
                        psl = slice(q2 * 512, (q2 + 1) * 512)
                        nc.tensor.matmul(pp[:, psl], lhsT=wT[(name, 0)], rhs=xs0[:, sl], start=True, stop=False)
                        nc.tensor.matmul(pp[:, psl], lhsT=wT[(name, 1)], rhs=xs1[:, sl], start=False, stop=False)
                        nc.tensor.matmul(pp[:, psl], lhsT=augT[name], rhs=aug2[:, sl], start=False, stop=True)
                    nc.vector.tensor_copy(dst[:, hsl], pp)
            vsb = big.tile([P, NE, HID], f16, tag='vsb')

            def emit_v_chunk(e):
                se = slice(e * P, (e + 1) * P)
                vp = psA.tile([P, HID], f32, tag='psA', name=f'vp{e}')
                nc.tensor.matmul(vp, lhsT=xs0[:, se], rhs=wT[('v', 0)], start=True, stop=False)
                nc.tensor.matmul(vp, lhsT=xs1[:, se], rhs=wT[('v', 1)], start=False, stop=False)
                nc.tensor.matmul(vp, lhsT=aug2[:, se], rhs=augT['v'], start=False, stop=True)
                nc.vector.tensor_copy(vsb[:, e, :], vp)

            # ---- attention: pipelined over (d, e); ACT exp stream is the roofline ----
            def emit_qk_exp(d, e):
                sd = slice(d * DQ, (d + 1) * DQ)
                se = slice(e * P, (e + 1) * P)
                ats = []
                for half in range(2):
                    sp = psA.tile([P, 1024], f32, tag='psA', name=f'sp{d}_{e}_{half}')
                    for hh in range(2):
                        h = half * 2 + hh
                        hp = slice(32 * h, 32 * h + 32)
                        nc.tensor.matmul(sp[:, hh * DQ:(hh + 1) * DQ], lhsT=kt[hp, se],
                                         rhs=qt[hp, sd], start=True, stop=True,
                                         tile_position=(32 * h, 0))
                    at = apool.tile([P, 1024], f16, tag='at', name=f'at{d}_{e}_{half}')
                    nc.scalar.activation(at, sp, AF.Exp, scale=SCALE)
                    ats.append(at)
                return ats

            def emit_zav(d, e, ats, zp, op_):
                for half in range(2):
                    at = ats[half]
                    for hh in range(2):
                        h = half * 2 + hh
                        hp = slice(32 * h, 32 * h + 32)
                        asl = slice(hh * DQ, (hh + 1) * DQ)
                        nc.tensor.matmul(zp[hp, :], lhsT=ones16[:, 0:32], rhs=at[:, asl],
                                         start=(e == 0), stop=(e == NE - 1),
                                         tile_position=(0, 32 * h))
                        nc.tensor.matmul(op_[hp, :], lhsT=vsb[:, e, hp], rhs=at[:, asl],
                                         start=(e == 0), stop=(e == NE - 1),
                                         tile_position=(0, 32 * h))

            def emit_norm_tail(d, zp, op_):
                rz = tpool.tile([P, DQ], f32, tag='rz', name=f'rz{d}')
                nc.vector.reciprocal(rz, zp)
                onorm = tpool.tile([P, DQ], f16, tag='onorm', name=f'onorm{d}')
                nc.vector.tensor_mul(onorm, op_, rz)
                return onorm

            def emit_proj_tail(d, onorm):
                sd = slice(d * DQ, (d + 1) * DQ)
                yp = psA.tile([P, 1024], f32, tag='psA', name=f'yp{d}')
                for c in range(2):
                    nc.tensor.matmul(yp[:, c * DQ:(c + 1) * DQ], lhsT=woT[:, c * P:(c + 1) * P],
                                     rhs=onorm, start=True, stop=True)
                for c in range(2):
                    ysb = tpool.tile([P, DQ], f32, tag='ysb', name=f'ysb{d}_{c}')
                    xc = x0 if c == 0 else x1
                    nc.vector.scalar_tensor_tensor(ysb, in0=yp[:, c * DQ:(c + 1) * DQ],
                                                   scalar=boc[c], in1=xc[:, sd],
                                                   op0=OP.add, op1=OP.add)
                    nc.sync.dma_start(out=yd[c * P:(c + 1) * P, sd], in_=ysb)

            prev = None          # (d, e, ats) whose ZAV is emitted one position late
            zp = op_ = None      # PSUM accumulators of prev's d
            pending_norm = None  # (d, onorm) waiting for its out-projection
            for d in range(ND):
                for e in range(NE):
                    ats = emit_qk_exp(d, e)
                    if d == 0:
                        emit_v_chunk(e)
                    if prev is not None:
                        emit_zav(prev[0], prev[1], prev[2], zp, op_)
                        if prev[1] == NE - 1:
                            pending_norm = (prev[0], emit_norm_tail(prev[0], zp, op_))
                    if e == 0:
                        # allocate this d's accumulators AFTER the previous d's
                        # reciprocal was emitted (correct WAR ordering on psB)
                        zp = psB.tile([P, DQ], f32, tag='acc', name=f'zp{d}')
                        op_ = psB.tile([P, DQ], f32, tag='acc', name=f'op{d}')
                    elif pending_norm is not None:
                        emit_proj_tail(pending_norm[0], pending_norm[1])
                        pending_norm = None
                    prev = (d, e, ats)
            # drain
            emit_zav(prev[0], prev[1], prev[2], zp, op_)
            onorm_last = emit_norm_tail(ND - 1, zp, op_)
            emit_proj_tail(ND - 1, onorm_last)

    nc.compile()
    return nc


def _get_nc():
    global _cached
    if _cached is None:
        _cached = _build()
    return _cached


def kernel(**inputs):
    from concourse.bass_utils import run_bass_kernel_spmd

    x = np.ascontiguousarray(np.asarray(inputs['x'], dtype=np.float32))
    g = np.asarray(inputs['g'], dtype=np.float32).reshape(C, 1)
    b = np.asarray(inputs['b'], dtype=np.float32).reshape(C, 1)
    wq = np.ascontiguousarray(np.asarray(inputs['Wq'], dtype=np.float32))
    wk = np.ascontiguousarray(np.asarray(inputs['Wk'], dtype=np.float32))
    wv = np.ascontiguousarray(np.asarray(inputs['Wv'], dtype=np.float32))
    wo = np.ascontiguousarray(np.asarray(inputs['Wo'], dtype=np.float32))
    bo = np.asarray(inputs['bo'], dtype=np.float32).reshape(C, 1)

    nc = _get_nc()
    in_maps = [
        {'x': x[i], 'g': g, 'b': b, 'Wq': wq, 'Wk': wk, 'Wv': wv, 'Wo': wo, 'bo': bo}
        for i in range(B)
    ]
    res = run_bass_kernel_spmd(nc, in_maps, list(range(B)))
    return np.stack([res.results[i]['y'] for i in range(B)]).astype(np.float32)


# revision 17
# speedup vs baseline: 1.2473x; 1.0055x over previous
"""Trainium2 Bass kernel for CrossAttention (LayerNorm + self-attention + 1x1 conv + residual).

Sharding: data-parallel over batch - B=8, one batch element per NeuronCore.

Per-core design:
 - ScalarE exp stream is the roofline (H*L^2 = 16.8M exps @ 1 elem/cycle/lane
   ~= 133us); everything else is pipelined underneath it.
 - all matmuls fp16 (1 cycle/row on PE), fp32 PSUM accumulation.
 - LayerNorm folded into QKV projections via augmented contraction rows;
   softmax without max-subtraction (logits are O(1)).
 - scores for 2 heads x 512 queries per [128,1024] PSUM tile, 3-deep rotation;
   Z/AV matmuls run one pipeline position behind the exps.
 - a dead ~5us chained-matmul warmup burst runs during the LayerNorm DVE/ACT
   chain so the PE HAM clock-gate opens (1.2 -> 2.4 GHz) before the attention
   stream starts; the attention keeps PE gaps well under the ~3.4us MID window
   so it never re-throttles.
 - single ACT table load: the ln/exp loads are steered to the combined
   natural_log_exp_and_others set (see _patch_act_tables).
"""
import numpy as np

B, C, L = 8, 256, 2048
H, DH = 4, 32
HID = H * DH           # 128
EPS = 1e-5
SCALE = DH ** -0.5
P = 128                # partitions
DQ = 512               # query tile (free dim of score matmuls)
ND = L // DQ           # 4 d-tiles
NE = L // P            # 16 key tiles of 128

_cached = None


def _patch_act_tables():
    """Steer the greedy ACT-table-load inserter to the combined ln+exp set.

    The inserter picks the first act_func_set containing the needed function;
    'exp' hits exp_and_others and 'ln' hits natural_log, so an interleaved
    ln/exp sequence reloads tables on every switch (1.28us each). Dropping
    those functions from the other sets (indices preserved) makes both
    resolve to natural_log_exp_and_others -> one load for the whole kernel.
    Set ids are positional, so only contents may be edited.
    """
    import concourse.bacc as bacc

    if getattr(bacc, '_act_tables_patched', False):
        return
    orig = bacc.get_activation_tables

    def patched(arch):
        tables = {k: set(v) for k, v in orig(arch).items()}
        if 'natural_log_exp_and_others' in tables:
            combined = tables['natural_log_exp_and_others']
            for name, fns in tables.items():
                if name != 'natural_log_exp_and_others':
                    fns -= {f for f in fns if f in combined and str(f).lower().endswith(('exp', 'ln'))}
        return tables

    bacc.get_activation_tables = patched
    bacc._act_tables_patched = True


def _build():
    import concourse.bass as bass
    import concourse.bacc as bacc
    import concourse.tile as tile
    from concourse import mybir
    from concourse.masks import make_identity

    _patch_act_tables()

    f32 = mybir.dt.float32
    f16 = mybir.dt.float16
    AF = mybir.ActivationFunctionType
    OP = mybir.AluOpType

    nc = bacc.Bacc('TRN2', target_bir_lowering=False, debug=False, num_devices=B)

    xd = nc.dram_tensor('x', [C, L], f32, kind='ExternalInput').ap()
    gd = nc.dram_tensor('g', [C, 1], f32, kind='ExternalInput').ap()
    bd = nc.dram_tensor('b', [C, 1], f32, kind='ExternalInput').ap()
    wqd = nc.dram_tensor('Wq', [HID, C], f32, kind='ExternalInput').ap()
    wkd = nc.dram_tensor('Wk', [HID, C], f32, kind='ExternalInput').ap()
    wvd = nc.dram_tensor('Wv', [HID, C], f32, kind='ExternalInput').ap()
    wod = nc.dram_tensor('Wo', [C, HID], f32, kind='ExternalInput').ap()
    bod = nc.dram_tensor('bo', [C, 1], f32, kind='ExternalInput').ap()
    yd = nc.dram_tensor('y', [C, L], f32, kind='ExternalOutput').ap()
    wsd = nc.dram_tensor('wscr', [1, 1], f32, kind='ExternalOutput').ap()

    with tile.TileContext(nc) as tc:
        with (
            tc.tile_pool(name='const', bufs=1) as const,
            tc.tile_pool(name='big', bufs=1) as big,
            tc.tile_pool(name='sc', bufs=2) as sc,
            tc.tile_pool(name='apool', bufs=4) as apool,
            tc.tile_pool(name='tpool', bufs=2) as tpool,
            tc.tile_pool(name='psA', bufs=3, space='PSUM') as psA,
            tc.tile_pool(name='psB', bufs=2, space='PSUM') as psB,
        ):
            wq_nat = const.tile([HID, C], f32, tag='wq_nat')
            wk_nat = const.tile([HID, C], f32, tag='wk_nat')
            wv_nat = const.tile([HID, C], f32, tag='wv_nat')
            wo_nat = [const.tile([P, HID], f32, tag=f'wo_nat{c}', name=f'wo_nat{c}') for c in range(2)]
            nc.sync.dma_start(out=wq_nat, in_=wqd)
            nc.sync.dma_start(out=wk_nat, in_=wkd)
            nc.sync.dma_start(out=wv_nat, in_=wvd)
            for c in range(2):
                nc.sync.dma_start(out=wo_nat[c], in_=wod[c * P:(c + 1) * P, :])

            gc = [const.tile([P, 1], f32, tag=f'g{c}', name=f'g{c}') for c in range(2)]
            bc = [const.tile([P, 1], f32, tag=f'b{c}', name=f'b{c}') for c in range(2)]
            boc = [const.tile([P, 1], f32, tag=f'bo{c}', name=f'bo{c}') for c in range(2)]
            for c in range(2):
                nc.sync.dma_start(out=gc[c], in_=gd[c * P:(c + 1) * P, :])
                nc.sync.dma_start(out=bc[c], in_=bd[c * P:(c + 1) * P, :])
                nc.sync.dma_start(out=boc[c], in_=bod[c * P:(c + 1) * P, :])

            # x loads after the (small) weight DMAs: weight prep + PE warmup
            # depend on the weights, LN stats start once the first chunks land
            x0 = big.tile([P, L], f32, tag='x0')
            x1 = big.tile([P, L], f32, tag='x1')
            for n in range(4):
                sl = slice(n * 512, (n + 1) * 512)
                nc.sync.dma_start(out=x0[:, sl], in_=xd[0:P, sl])
                nc.sync.dma_start(out=x1[:, sl], in_=xd[P:C, sl])

            ident = const.tile([P, P], f32, tag='ident')
            make_identity(nc, ident)
            ones16 = const.tile([P, P], f16, tag='ones16')
            nc.gpsimd.memset(ones16, 1.0)
            epst = const.tile([P, 1], f32, tag='epst')
            nc.vector.memset(epst, EPS)

            # ---- weight prep: wT = (W scaled by g)^T in fp16, aug rows ----
            wT = {}
            for name, nat in (('q', wq_nat), ('k', wk_nat), ('v', wv_nat)):
                for c in range(2):
                    tp = psA.tile([P, P], f32, tag='psA', name=f'tp_{name}{c}')
                    nc.tensor.transpose(tp, nat[:, c * P:(c + 1) * P], ident)
                    t = const.tile([P, HID], f16, tag=f'w{name}T{c}', name=f'w{name}T{c}')
                    nc.vector.tensor_copy(t, tp)
                    wT[(name, c)] = t
            woT = const.tile([HID, C], f16, tag='woT')
            for c in range(2):
                tp = psA.tile([P, P], f32, tag='psA', name=f'tp_wo{c}')
                nc.tensor.transpose(tp, wo_nat[c], ident)
                nc.vector.tensor_copy(woT[:, c * P:(c + 1) * P], tp)

            # ---- PE warmup: ~4.5us of chained dead matmuls, pinned here so it
            # runs during the x-DMA/LayerNorm head; the HAM clock gate opens
            # (1.2 -> 2.4 GHz) before the QKV/attention stream, and the
            # attention pipeline keeps PE gaps well under the ~3.4us MID
            # window so it never re-throttles ----
            warm_src = sc.tile([P, 512], f16, tag='wsrc')
            nc.vector.tensor_copy(warm_src, x0[:, 0:512])
            warm_ps = psB.tile([P, 512], f32, tag='acc', name='warm_ps')
            NWARM = 10
            for i in range(NWARM):
                nc.tensor.matmul(warm_ps, lhsT=ones16, rhs=warm_src,
                                 start=(i == 0), stop=(i == NWARM - 1))
            wscr_sb = sc.tile([1, 1], f32, tag='wscr_sb')
            nc.vector.tensor_copy(wscr_sb, warm_ps[0:1, 0:1])
            nc.sync.dma_start(out=wsd, in_=wscr_sb)

            # augmentation rows: row0 = -s_g (for mean), row1 = b_proj
            augin = []
            for c in range(2):
                ai = const.tile([P, 2], f16, tag=f'augin{c}', name=f'augin{c}')
                nc.vector.tensor_scalar_mul(ai[:, 0:1], gc[c], -1.0)
                nc.vector.tensor_copy(ai[:, 1:2], bc[c])
                augin.append(ai)
            augT = {}
            for name in ('q', 'k', 'v'):
                ap_ = psA.tile([2, P], f32, tag='psA', name=f'augps{name}')
                for c in range(2):
                    nc.tensor.matmul(ap_, lhsT=augin[c], rhs=wT[(name, c)],
                                     start=(c == 0), stop=(c == 1))
                t = const.tile([2, P], f16, tag=f'augT{name}', name=f'augT{name}')
                nc.vector.tensor_copy(t, ap_)
                augT[name] = t
            for name in ('q', 'k', 'v'):
                for c in range(2):
                    nc.vector.tensor_scalar_mul(wT[(name, c)], wT[(name, c)], gc[c])

            # ---- LayerNorm stats via fp16 ones-matmuls (replicated on 128 parts) ----
            mean_bc = big.tile([P, L], f32, tag='mean')
            rstd_bc = big.tile([P, L], f32, tag='rstd')
            xs0 = big.tile([P, L], f16, tag='xs0')
            xs1 = big.tile([P, L], f16, tag='xs1')
            aug2 = big.tile([2, L], f16, tag='aug2')
            nc.gpsimd.memset(aug2, 1.0)

            for hl in range(2):
                hsl = slice(hl * 1024, (hl + 1) * 1024)
                s1h = psA.tile([P, 1024], f32, tag='psA', name=f's1h{hl}')
                s2h = psA.tile([P, 1024], f32, tag='psA', name=f's2h{hl}')
                for q2 in range(2):
                    sl = slice(hl * 1024 + q2 * 512, hl * 1024 + (q2 + 1) * 512)
                    psl = slice(q2 * 512, (q2 + 1) * 512)
                    for c, xc in ((0, x0), (1, x1)):
                        xb = sc.tile([P, 512], f16, tag='xb')
                        nc.vector.tensor_copy(xb, xc[:, sl])
                        xsq = sc.tile([P, 512], f16, tag='xsq')
                        nc.gpsimd.tensor_mul(xsq, xc[:, sl], xc[:, sl])
                        nc.tensor.matmul(s1h[:, psl], lhsT=ones16, rhs=xb,
                                         start=(c == 0), stop=(c == 1))
                        nc.tensor.matmul(s2h[:, psl], lhsT=ones16, rhs=xsq,
                                         start=(c == 0), stop=(c == 1))
                nc.vector.tensor_scalar_mul(mean_bc[:, hsl], s1h, 1.0 / C)
                msq = sc.tile([P, 1024], f32, tag='msq')
                nc.vector.tensor_mul(msq, mean_bc[:, hsl], mean_bc[:, hsl])
                veps = sc.tile([P, 1024], f32, tag='veps')
                nc.vector.scalar_tensor_tensor(veps, in0=s2h, scalar=1.0 / C, in1=msq,
                                               op0=OP.mult, op1=OP.subtract)
                lv = sc.tile([P, 1024], f32, tag=f'lnv{hl}', name=f'lnv{hl}')
                nc.scalar.activation(lv, veps, AF.Ln, bias=epst)
                # rstd = exp(-0.5*ln(var+eps)); single combined ln/exp ACT table
                nc.scalar.activation(rstd_bc[:, hsl], lv, AF.Exp, scale=-0.5)
                nc.vector.tensor_mul(xs0[:, hsl], x0[:, hsl], rstd_bc[:, hsl])
                nc.vector.tensor_mul(xs1[:, hsl], x1[:, hsl], rstd_bc[:, hsl])
                nc.vector.tensor_mul(aug2[0:1, hsl], mean_bc[0:1, hsl], rstd_bc[0:1, hsl])

            # ---- QKV projections: qt/kt feature-major [HID, L] ----
            qt = big.tile([HID, L], f16, tag='qt')
            kt = big.tile([HID, L], f16, tag='kt')
            for hl in range(2):
                hsl = slice(hl * 1024, (hl + 1) * 1024)
                for name, dst in (('q', qt), ('k', kt)):
                    pp = psA.tile([P, 1024], f32, tag='psA', name=f'pp{name}{hl}')
                    for q2 in range(2):
                        sl = slice(hl * 1024 + q2 * 512, hl * 1024 + (q2 + 1) * 512)
                        psl = slice(q2 * 512, (q2 + 1) * 512)
                        nc.tensor.matmul(pp[:, psl], lhsT=wT[(name, 0)], rhs=xs0[:, sl], start=True, stop=False)
                        nc.tensor.matmul(pp[:, psl], lhsT=wT[(name, 1)], rhs=xs1[:, sl], start=False, stop=False)
                        nc.tensor.matmul(pp[:, psl], lhsT=augT[name], rhs=aug2[:, sl], start=False, stop=True)
                    nc.vector.tensor_copy(dst[:, hsl], pp)
            vsb = big.tile([P, NE, HID], f16, tag='vsb')

            def emit_v_chunk(e):
                se = slice(e * P, (e + 1) * P)
                vp = psA.tile([P, HID], f32, tag='psA', name=f'vp{e}')
                nc.tensor.matmul(vp, lhsT=xs0[:, se], rhs=wT[('v', 0)], start=True, stop=False)
                nc.tensor.matmul(vp, lhsT=xs1[:, se], rhs=wT[('v', 1)], start=False, stop=False)
                nc.tensor.matmul(vp, lhsT=aug2[:, se], rhs=augT['v'], start=False, stop=True)
                nc.vector.tensor_copy(vsb[:, e, :], vp)

            # ---- attention: pipelined over (d, e); ACT exp stream is the roofline ----
            def emit_qk_exp(d, e):
                sd = slice(d * DQ, (d + 1) * DQ)
                se = slice(e * P, (e + 1) * P)
                ats = []
                for half in range(2):
                    sp = psA.tile([P, 1024], f32, tag='psA', name=f'sp{d}_{e}_{half}')
                    for hh in range(2):
                        h = half * 2 + hh
                        hp = slice(32 * h, 32 * h + 32)
                        nc.tensor.matmul(sp[:, hh * DQ:(hh + 1) * DQ], lhsT=kt[hp, se],
                                         rhs=qt[hp, sd], start=True, stop=True,
                                         tile_position=(32 * h, 0))
                    at = apool.tile([P, 1024], f16, tag='at', name=f'at{d}_{e}_{half}')
                    nc.scalar.activation(at, sp, AF.Exp, scale=SCALE)
                    ats.append(at)
                return ats

            def emit_zav(d, e, ats, zp, op_):
                for half in range(2):
                    at = ats[half]
                    for hh in range(2):
                        h = half * 2 + hh
                        hp = slice(32 * h, 32 * h + 32)
                        asl = slice(hh * DQ, (hh + 1) * DQ)
                        nc.tensor.matmul(zp[hp, :], lhsT=ones16[:, 0:32], rhs=at[:, asl],
                                         start=(e == 0), stop=(e == NE - 1),
                                         tile_position=(0, 32 * h))
                        nc.tensor.matmul(op_[hp, :], lhsT=vsb[:, e, hp], rhs=at[:, asl],
                                         start=(e == 0), stop=(e == NE - 1),
                                         tile_position=(0, 32 * h))

            def emit_norm_tail(d, zp, op_):
                rz = tpool.tile([P, DQ], f32, tag='rz', name=f'rz{d}')
                nc.vector.reciprocal(rz, zp)
                onorm = tpool.tile([P, DQ], f16, tag='onorm', name=f'onorm{d}')
                nc.vector.tensor_mul(onorm, op_, rz)
                return onorm

            def emit_proj_tail(d, onorm):
                sd = slice(d * DQ, (d + 1) * DQ)
                yp = psA.tile([P, 1024], f32, tag='psA', name=f'yp{d}')
                for c in range(2):
                    nc.tensor.matmul(yp[:, c * DQ:(c + 1) * DQ], lhsT=woT[:, c * P:(c + 1) * P],
                                     rhs=onorm, start=True, stop=True)
                for c in range(2):
                    ysb = tpool.tile([P, DQ], f32, tag='ysb', name=f'ysb{d}_{c}')
                    xc = x0 if c == 0 else x1
                    nc.vector.scalar_tensor_tensor(ysb, in0=yp[:, c * DQ:(c + 1) * DQ],
                                                   scalar=boc[c], in1=xc[:, sd],
                                                   op0=OP.add, op1=OP.add)
                    nc.sync.dma_start(out=yd[c * P:(c + 1) * P, sd], in_=ysb)

            prev = None          # (d, e, ats) whose ZAV is emitted one position late
            zp = op_ = None      # PSUM accumulators of prev's d
            pending_norm = None  # (d, onorm) waiting for its out-projection
            for d in range(ND):
                for e in range(NE):
                    ats = emit_qk_exp(d, e)
                    if d == 0:
                        emit_v_chunk(e)
                    if prev is not None:
                        emit_zav(prev[0], prev[1], prev[2], zp, op_)
                        if prev[1] == NE - 1:
                            pending_norm = (prev[0], emit_norm_tail(prev[0], zp, op_))
                    if e == 0:
                        # allocate this d's accumulators AFTER the previous d's
                        # reciprocal was emitted (correct WAR ordering on psB)
                        zp = psB.tile([P, DQ], f32, tag='acc', name=f'zp{d}')
                        op_ = psB.tile([P, DQ], f32, tag='acc', name=f'op{d}')
                    elif pending_norm is not None:
                        emit_proj_tail(pending_norm[0], pending_norm[1])
                        pending_norm = None
                    prev = (d, e, ats)
            # drain
            emit_zav(prev[0], prev[1], prev[2], zp, op_)
            onorm_last = emit_norm_tail(ND - 1, zp, op_)
            emit_proj_tail(ND - 1, onorm_last)

    nc.compile()
    return nc


def _get_nc():
    global _cached
    if _cached is None:
        _cached = _build()
    return _cached


def kernel(**inputs):
    from concourse.bass_utils import run_bass_kernel_spmd

    x = np.ascontiguousarray(np.asarray(inputs['x'], dtype=np.float32))
    g = np.asarray(inputs['g'], dtype=np.float32).reshape(C, 1)
    b = np.asarray(inputs['b'], dtype=np.float32).reshape(C, 1)
    wq = np.ascontiguousarray(np.asarray(inputs['Wq'], dtype=np.float32))
    wk = np.ascontiguousarray(np.asarray(inputs['Wk'], dtype=np.float32))
    wv = np.ascontiguousarray(np.asarray(inputs['Wv'], dtype=np.float32))
    wo = np.ascontiguousarray(np.asarray(inputs['Wo'], dtype=np.float32))
    bo = np.asarray(inputs['bo'], dtype=np.float32).reshape(C, 1)

    nc = _get_nc()
    in_maps = [
        {'x': x[i], 'g': g, 'b': b, 'Wq': wq, 'Wk': wk, 'Wv': wv, 'Wo': wo, 'bo': bo}
        for i in range(B)
    ]
    res = run_bass_kernel_spmd(nc, in_maps, list(range(B)))
    return np.stack([res.results[i]['y'] for i in range(B)]).astype(np.float32)
